# revision 1
# baseline (speedup 1.0000x reference)
"""GatedGraphConvolution Trainium2 kernel.

out = input + segment_sum(sigmoid(g) * e, edge_sources)
  where [g|e] = input[edge_targets] @ W.T

Key algebraic fact: the per-edge message depends ONLY on the target node:
  msg_e = M[target_e],  M[n] = sigmoid(x_n @ Wg.T) * (x_n @ We.T)
so we (phase A) compute the dense M table [N, F] once per core, and
(phase B) gather M rows per edge + scatter-add by source via one-hot
matmuls accumulated in PSUM.

Sharding: nodes are sharded by SOURCE across the 8 cores (6250 nodes each);
each core receives exactly the edges whose source is in its slice, so there
are no collectives.  Edges are sorted by 128-node source "window"; each
window's edges accumulate into one PSUM tile via lhsT=onehot matmuls.
The dma_gather int16 index limit (<=32767) is handled by splitting each
window's edges into low-target (< 32768) and high-target tiles and issuing
gathers against two base offsets of the M table.
"""

import math
import os
import sys
from dataclasses import dataclass, field

import numpy as np

if "/opt/trn_rl_repo" not in sys.path:
    sys.path.insert(0, "/opt/trn_rl_repo")

import ml_dtypes

P = 128  # partitions / tile edge
F = 128  # feature dim (OUT_F == IN_F == 128)
TF = 2 * F

BF16 = ml_dtypes.bfloat16


@dataclass
class Cfg:
    n_nodes: int = 50000
    n_cores: int = 8
    half: int = 32768  # int16 index limit boundary (multiple of 128)
    gw: int = 5  # windows per gather group
    ca: int = 16  # node-tiles per phase-A chunk

    @property
    def na(self) -> int:  # node tiles in M table
        return math.ceil(self.n_nodes / P)

    @property
    def npc(self) -> int:  # nodes per core
        assert self.n_nodes % self.n_cores == 0
        return self.n_nodes // self.n_cores

    @property
    def nwin(self) -> int:  # source windows per core
        return math.ceil(self.npc / P)


@dataclass
class Plan:
    """Static schedule shared by all cores + per-core host arrays."""

    T_lo: list  # tiles per (window, low-half), max over cores
    T_hi: list
    groups: list = field(default_factory=list)  # (ws, lo_tiles, hi_tiles)
    tiles_of: dict = field(default_factory=dict)  # (w, half) -> [tile ids]
    win_tiles: dict = field(default_factory=dict)  # w -> [(tile, half, pos_in_buf)]
    TT: int = 0
    # per-core packed arrays
    srel: list = field(default_factory=list)  # [P, TT] bf16
    gidx: list = field(default_factory=list)  # [P, 8*TT] int16


def _make_schedule(cfg: Cfg, T_lo, T_hi) -> Plan:
    plan = Plan(T_lo=T_lo, T_hi=T_hi)
    t = 0
    for g0 in range(0, cfg.nwin, cfg.gw):
        ws = list(range(g0, min(g0 + cfg.gw, cfg.nwin)))
        lo, hi = [], []
        for w in ws:
            for _ in range(T_lo[w]):
                plan.tiles_of.setdefault((w, 0), []).append(t)
                lo.append((w, t))
                t += 1
        for w in ws:
            for _ in range(T_hi[w]):
                plan.tiles_of.setdefault((w, 1), []).append(t)
                hi.append((w, t))
                t += 1
        plan.groups.append((ws, lo, hi))
    plan.TT = t
    # per window: list of (global tile id, half, position within the group's
    # lo/hi gather buffer) in mm2 consumption order
    for ws, lo, hi in plan.groups:
        for w in ws:
            lst = []
            for pos, (w2, t2) in enumerate(lo):
                if w2 == w:
                    lst.append((t2, 0, pos))
            for pos, (w2, t2) in enumerate(hi):
                if w2 == w:
                    lst.append((t2, 1, pos))
            plan.win_tiles[w] = lst
    return plan


def _plan(cfg: Cfg, edge_sources: np.ndarray, edge_targets: np.ndarray) -> Plan:
    src = edge_sources.astype(np.int64)
    tgt = edge_targets.astype(np.int64)
    npc, nwin = cfg.npc, cfg.nwin

    # bucket edges per (core, window, half)
    core = src // npc
    w_all = (src % npc) // P
    srel_all = (src % npc) % P
    hi_all = (tgt >= cfg.half).astype(np.int64)

    counts = np.zeros((cfg.n_cores, nwin, 2), np.int64)
    np.add.at(counts, (core, w_all, hi_all), 1)
    tmax = counts.max(axis=0)  # [nwin, 2]
    T_lo = [int(math.ceil(tmax[w, 0] / P)) for w in range(nwin)]
    T_hi = [int(math.ceil(tmax[w, 1] / P)) for w in range(nwin)]

    plan = _make_schedule(cfg, T_lo, T_hi)

    # pack per-core slot arrays
    order = np.lexsort((hi_all, w_all, core))
    src_s, w_s, srel_s, hi_s, tgt_s = (
        src[order],
        w_all[order],
        srel_all[order],
        hi_all[order],
        tgt[order],
    )
    bounds = {}
    keys = core[order] * (nwin * 2) + w_s * 2 + hi_s
    uniq, starts = np.unique(keys, return_index=True)
    starts = list(starts) + [len(keys)]
    for i, k in enumerate(uniq):
        bounds[int(k)] = (starts[i], starts[i + 1])

    for c in range(cfg.n_cores):
        srel_arr = np.full((plan.TT * P,), 255.0, np.float32)
        gidx_arr = np.zeros((plan.TT * P,), np.int16)
        for w in range(nwin):
            for h in (0, 1):
                k = c * (nwin * 2) + w * 2 + h
                if k not in bounds:
                    continue
                a, b = bounds[k]
                tiles = plan.tiles_of.get((w, h), [])
                assert (b - a) <= len(tiles) * P
                sr = srel_s[a:b]
                tg = tgt_s[a:b] - (cfg.half if h else 0)
                for i in range(b - a):
                    t = tiles[i // P]
                    j = i % P
                    s = t * P + j
                    srel_arr[s] = sr[i]
                    gidx_arr[s] = tg[i]
        srel_host = srel_arr.reshape(plan.TT, P).T.astype(BF16)  # [P, TT]
        g16 = gidx_arr.reshape(plan.TT * 8, 16).T  # [16, TT*8]
        gidx_host = np.tile(g16, (8, 1)).astype(np.int16)  # [P, TT*8]
        plan.srel.append(np.ascontiguousarray(srel_host))
        plan.gidx.append(np.ascontiguousarray(gidx_host))
    return plan


def _host_arrays(cfg: Cfg, inp: np.ndarray, W: np.ndarray):
    """Replicated input arrays: xT (transposed node features), wT, iota."""
    n = cfg.n_nodes
    xT = np.zeros((P, cfg.na * P), np.float32)
    xT[:, :n] = inp.T
    wT = np.ascontiguousarray(W.T)  # [F, 2F]
    iota = np.tile(np.arange(P, dtype=np.float32), (P, 1))
    return (
        np.ascontiguousarray(xT.astype(BF16)),
        np.ascontiguousarray(wT.astype(BF16)),
        np.ascontiguousarray(iota.astype(BF16)),
    )


def _xs_core(cfg: Cfg, inp: np.ndarray, c: int) -> np.ndarray:
    """Per-core input slice packed [P, nwin*F]: xs[p, w*F+f] = x[c*npc+w*P+p, f]."""
    npc, nwin = cfg.npc, cfg.nwin
    sl = np.zeros((nwin * P, F), np.float32)
    sl[:npc] = inp[c * npc : (c + 1) * npc]
    return np.ascontiguousarray(sl.reshape(nwin, P, F).transpose(1, 0, 2).reshape(P, nwin * F))


def _build(cfg: Cfg, plan: Plan, enable_asserts: bool = False):
    import concourse.bacc as bacc
    import concourse.tile as tile
    from concourse import mybir

    nc = bacc.Bacc(
        "TRN2",
        target_bir_lowering=False,
        debug=False,
        enable_asserts=enable_asserts,
        num_devices=cfg.n_cores,
    )
    dt = mybir.dt

    na, nwin, npc = cfg.na, cfg.nwin, cfg.npc
    TT = plan.TT

    xT_d = nc.dram_tensor("xT", [P, na * P], dt.bfloat16, kind="ExternalInput")
    wT_d = nc.dram_tensor("wT", [P, TF], dt.bfloat16, kind="ExternalInput")
    iota_d = nc.dram_tensor("iota", [P, P], dt.bfloat16, kind="ExternalInput")
    xs_d = nc.dram_tensor("xs", [P, nwin * F], dt.float32, kind="ExternalInput")
    srel_d = nc.dram_tensor("srel", [P, TT], dt.bfloat16, kind="ExternalInput")
    gidx_d = nc.dram_tensor("gidx", [P, 8 * TT], dt.int16, kind="ExternalInput")
    y_d = nc.dram_tensor("y", [npc, F], dt.float32, kind="ExternalOutput")
    mtab_d = nc.dram_tensor("mtab", [na * P, F], dt.bfloat16, kind="Internal")

    lo_rows = min(cfg.half, na * P)

    max_tl = max((len(lo) for _, lo, _ in plan.groups), default=0)
    max_th = max((len(hi) for _, _, hi in plan.groups), default=0)
    max_tg = max((len(lo) + len(hi) for _, lo, hi in plan.groups), default=0)

    n_chunks = math.ceil(na / cfg.ca)

    from concourse.tile import add_dep_helper

    with tile.TileContext(nc) as tc:
        import contextlib

        with contextlib.ExitStack() as ctx:
            consts = ctx.enter_context(tc.tile_pool(name="consts", bufs=1))
            a_in = ctx.enter_context(tc.tile_pool(name="a_in", bufs=3))
            a_ps = ctx.enter_context(tc.tile_pool(name="a_ps", bufs=4, space="PSUM"))
            a_sg = ctx.enter_context(tc.tile_pool(name="a_sg", bufs=4))
            a_m = ctx.enter_context(tc.tile_pool(name="a_m", bufs=3))
            b_lo = ctx.enter_context(tc.tile_pool(name="b_lo", bufs=2))
            b_hi = ctx.enter_context(tc.tile_pool(name="b_hi", bufs=2))
            b_oh = ctx.enter_context(tc.tile_pool(name="b_oh", bufs=2))
            b_ps = ctx.enter_context(tc.tile_pool(name="b_ps", bufs=2, space="PSUM"))
            b_out = ctx.enter_context(tc.tile_pool(name="b_out", bufs=2))

            # ---- constants to SBUF ----
            wT_sb = consts.tile([P, TF], dt.bfloat16, tag="wT")
            nc.sync.dma_start(wT_sb[:], wT_d[:, :])
            iota_sb = consts.tile([P, P], dt.bfloat16, tag="iota")
            nc.sync.dma_start(iota_sb[:], iota_d[:, :])
            xs_sb = consts.tile([P, nwin * F], dt.float32, tag="xs")
            nc.sync.dma_start(xs_sb[:], xs_d[:, :])
            srel_sb = consts.tile([P, TT], dt.bfloat16, tag="srel")
            nc.sync.dma_start(srel_sb[:], srel_d[:, :])
            gidx_sb = consts.tile([P, 8 * TT], dt.int16, tag="gidx")
            nc.sync.dma_start(gidx_sb[:], gidx_d[:, :])

            # ---- phase A: M table ----
            mdmas = []
            for ci in range(n_chunks):
                c0 = ci * cfg.ca
                ca = min(cfg.ca, na - c0)
                xt = a_in.tile([P, cfg.ca * P], dt.bfloat16, tag="xt")
                nc.sync.dma_start(xt[:, : ca * P], xT_d[:, c0 * P : (c0 + ca) * P])
                mtile = a_m.tile([P, cfg.ca * F], dt.bfloat16, tag="mtile")
                for k in range(ca):
                    ps = a_ps.tile([P, TF], dt.float32, tag="psA")
                    nc.tensor.matmul(
                        ps[:],
                        lhsT=xt[:, k * P : (k + 1) * P],
                        rhs=wT_sb[:],
                        start=True,
                        stop=True,
                    )
                    sg = a_sg.tile([P, F], dt.float32, tag="sg")
                    nc.scalar.activation(
                        sg[:], ps[:, 0:F], mybir.ActivationFunctionType.Sigmoid
                    )
                    nc.vector.tensor_mul(
                        mtile[:, k * F : (k + 1) * F], ps[:, F:TF], sg[:]
                    )
                out_ap = (
                    mtab_d[c0 * P : (c0 + ca) * P, :]
                    .rearrange("(k p) f -> p k f", p=P)
                )
                mdmas.append(
                    nc.sync.dma_start(
                        out_ap, mtile[:, : ca * F].rearrange("p (k f) -> p k f", f=F)
                    )
                )

            # ---- phase B: gather + one-hot scatter ----
            for ws, lo, hi in plan.groups:
                lob = hib = None
                if lo:
                    t0 = lo[0][1]
                    tl = len(lo)
                    lob = b_lo.tile([P, max(max_tl, 1) * F], dt.bfloat16, tag="lob")
                    g = nc.gpsimd.dma_gather(
                        out_ap=lob[:, : tl * F].rearrange("p (t e) -> p t e", e=F),
                        in_ap=mtab_d[0:lo_rows, :],
                        idxs_ap=gidx_sb[:, 8 * t0 : 8 * (t0 + tl)],
                        num_idxs=tl * P,
                        num_idxs_reg=tl * P,
                        elem_size=F,
                        single_packet=False,
                    )
                    for m in mdmas:
                        add_dep_helper(g.ins, m.ins, reason="mtab RAW")
                if hi:
                    t0 = hi[0][1]
                    th = len(hi)
                    hib = b_hi.tile([P, max(max_th, 1) * F], dt.bfloat16, tag="hib")
                    g = nc.gpsimd.dma_gather(
                        out_ap=hib[:, : th * F].rearrange("p (t e) -> p t e", e=F),
                        in_ap=mtab_d[cfg.half : na * P, :],
                        idxs_ap=gidx_sb[:, 8 * t0 : 8 * (t0 + th)],
                        num_idxs=th * P,
                        num_idxs_reg=th * P,
                        elem_size=F,
                        single_packet=False,
                    )
                    for m in mdmas:
                        add_dep_helper(g.ins, m.ins, reason="mtab RAW")

                # one-hot for the whole group in one DVE op
                tg0 = (lo + hi)[0][1] if (lo or hi) else None
                ntg = len(lo) + len(hi)
                oh = None
                if ntg:
                    oh = b_oh.tile([P, max(max_tg, 1) * P], dt.bfloat16, tag="oh")
                    nc.vector.tensor_tensor(
                        out=oh[:, : ntg * P].rearrange("p (t e) -> p t e", e=P),
                        in0=srel_sb[:, tg0 : tg0 + ntg]
                        .unsqueeze(2)
                        .to_broadcast([P, ntg, P]),
                        in1=iota_sb[:].unsqueeze(1).to_broadcast([P, ntg, P]),
                        op=mybir.AluOpType.is_equal,
                    )

                for w in ws:
                    tiles = plan.win_tiles.get(w, [])
                    rows = min(P, npc - w * P)
                    ot = b_out.tile([P, F], dt.float32, tag="ot")
                    if not tiles:
                        nc.vector.tensor_copy(ot[:], xs_sb[:, w * F : (w + 1) * F])
                    else:
                        ps = b_ps.tile([P, F], dt.float32, tag="psB")
                        for i, (t, h, pos) in enumerate(tiles):
                            buf = hib if h else lob
                            nc.tensor.matmul(
                                ps[:],
                                lhsT=oh[:, (t - tg0) * P : (t - tg0 + 1) * P],
                                rhs=buf[:, pos * F : (pos + 1) * F],
                                start=(i == 0),
                                stop=(i == len(tiles) - 1),
                            )
                        nc.vector.tensor_add(
                            ot[:], ps[:], xs_sb[:, w * F : (w + 1) * F]
                        )
                    nc.sync.dma_start(y_d[w * P : w * P + rows, :], ot[:rows, :])

    nc.compile()
    return nc


def _in_maps(cfg: Cfg, plan: Plan, inp: np.ndarray, W: np.ndarray):
    xT, wT, iota = _host_arrays(cfg, inp, W)
    maps = []
    for c in range(cfg.n_cores):
        maps.append(
            {
                "xT": xT,
                "wT": wT,
                "iota": iota,
                "xs": _xs_core(cfg, inp, c),
                "srel": plan.srel[c],
                "gidx": plan.gidx[c],
            }
        )
    return maps


def _install_ntff_hook():
    """Provide the antenv.axon_hooks shim trn_boot expects, so trace=True
    can capture NTFF profiles. Silently degrades if anything is missing."""
    try:
        import antenv.axon_hooks  # noqa: F401

        return
    except ImportError:
        pass
    try:
        import types

        import antenv

        mod = types.ModuleType("antenv.axon_hooks")
        _hook = [None]
        mod.set_axon_ntff_profile_hook = lambda h: _hook.__setitem__(0, h)
        mod.get_axon_ntff_profile_hook = lambda: _hook[0]
        sys.modules["antenv.axon_hooks"] = mod
        antenv.axon_hooks = mod
        from trn_agent_boot import trn_boot

        mod.set_axon_ntff_profile_hook(
            trn_boot._ntff_profile_via_ctypes("/opt/axon/libaxon_pjrt.so")
        )
    except Exception:
        pass


def kernel(**inputs) -> np.ndarray:
    inp = np.asarray(inputs["input"], np.float32)
    W = np.asarray(inputs["W"], np.float32)
    es = np.asarray(inputs["edge_sources"]).astype(np.int64)
    et = np.asarray(inputs["edge_targets"]).astype(np.int64)

    cfg = Cfg(n_nodes=inp.shape[0])
    plan = _plan(cfg, es, et)
    nc = _build(cfg, plan)

    from concourse.bass_utils import run_bass_kernel_spmd

    if bool(int(os.environ.get("GGC_TRACE", "0"))):
        _install_ntff_hook()
    res = run_bass_kernel_spmd(
        nc,
        _in_maps(cfg, plan, inp, W),
        core_ids=list(range(cfg.n_cores)),
        trace=bool(int(os.environ.get("GGC_TRACE", "0"))),
    )
    out = np.concatenate([res.results[c]["y"] for c in range(cfg.n_cores)], axis=0)
    if bool(int(os.environ.get("GGC_TRACE", "0"))):
        kernel.last_results = res  # stash for test harness
    return out



# revision 4
# speedup vs baseline: 1.6038x; 1.6038x over previous
"""GatedGraphConvolution Trainium2 kernel (v2).

out = input + segment_sum(sigmoid(g) * e, edge_sources)
  where [g|e] = input[edge_targets] @ W.T

Per-edge messages depend only on the target node:
  msg_e = M[target_e],  M[n] = sigmoid(x_n @ Wg.T) * (x_n @ We.T)
Phase A computes the dense M table once per core (replicated) into DRAM at a
512-byte row stride; phase B gathers M rows per edge with gpsimd dma_gather
and scatter-adds by source via one-hot matmuls accumulated in PSUM.

v2 speedups over v1:
 - dma_gather calls rotate over 4 SWDGE queues (descriptor rings on all four
   Q7 core pairs) -> ~3x gather throughput vs one queue.
 - single index space: mtab rows at 512B stride, signed int16 idx = tgt-25024
   (covers all 50048 rows; negative idx reach below the AP base stays inside
   the tensor). The v1 low/high-half split and its padding are gone.
 - per-core sources are bin-packed into 51 balanced windows of <=128 sources
   and exactly 16 edge-tiles (2048 edge slots); the host permutation is
   undone after the run. Uniform structure = shared SPMD program, ~10% fewer
   gathered rows than v1.
 - phase A sigmoid/mul run as chunk-wide strided ops (fewer, larger DVE/ACT
   instructions).
"""

import math
import os
import sys
from dataclasses import dataclass, field
from heapq import heapify, heappop, heappush

import numpy as np

if "/opt/trn_rl_repo" not in sys.path:
    sys.path.insert(0, "/opt/trn_rl_repo")

import ml_dtypes

P = 128  # partitions / tile edge
F = 128  # feature dim (OUT_F == IN_F == 128)
TF = 2 * F

BF16 = ml_dtypes.bfloat16

IDX_OFF = 25024  # gather idx = tgt - IDX_OFF, fits int16 for tgt in [0, 50048)


@dataclass
class Cfg:
    n_nodes: int = 50000
    n_cores: int = 8
    nwin: int = 51  # windows per core (each <=128 sources, 16 tiles)
    tpw: int = 16  # tiles per window
    tpc: int = 51  # tiles per gather chunk
    ca: int = 4  # node-tiles per phase-A chunk

    @property
    def na(self) -> int:  # node tiles in M table
        return math.ceil(self.n_nodes / P)

    @property
    def npc(self) -> int:  # nodes per core
        assert self.n_nodes % self.n_cores == 0
        return self.n_nodes // self.n_cores

    @property
    def ntiles(self) -> int:
        return self.nwin * self.tpw

    @property
    def nchunks(self) -> int:
        assert self.ntiles % self.tpc == 0
        return self.ntiles // self.tpc


@dataclass
class Plan:
    srel: list = field(default_factory=list)  # per-core [P, ntiles] bf16
    gidx: list = field(default_factory=list)  # per-core [P, 8*ntiles] int16
    xs: list = field(default_factory=list)  # per-core [P, nwin*F] fp32
    inv_rows: list = field(default_factory=list)  # per-core [npc] row in padded y


def _plan(cfg: Cfg, edge_sources: np.ndarray, edge_targets: np.ndarray, inp: np.ndarray) -> Plan:
    src = edge_sources.astype(np.int64)
    tgt = edge_targets.astype(np.int64)
    npc, nwin, tpw = cfg.npc, cfg.nwin, cfg.tpw
    cap_e = tpw * P  # 2048 edge slots per window
    plan = Plan()

    core = src // npc
    for c in range(cfg.n_cores):
        m = core == c
        e_src = (src[m] - c * npc).astype(np.int64)
        e_tgt = tgt[m]
        deg = np.bincount(e_src, minlength=npc)

        # LPT bin-packing: sources (heaviest first) onto windows, respecting
        # <=128 sources and <=2048 edges per window.
        order = np.argsort(-deg, kind="stable")
        heap = [(0, 0, w) for w in range(nwin)]  # (edges, nsrc, w)
        heapify(heap)
        win_of = np.empty(npc, np.int32)
        r_of = np.empty(npc, np.int32)
        overflow = []
        for s in order:
            d = int(deg[s])
            spill = []
            while True:
                e_w, n_w, w = heappop(heap)
                if n_w < P and e_w + d <= cap_e:
                    win_of[s] = w
                    r_of[s] = n_w
                    heappush(heap, (e_w + d, n_w + 1, w))
                    break
                spill.append((e_w, n_w, w))
                if not heap:
                    raise RuntimeError("binpack failed")
            for it in spill:
                heappush(heap, it)

        # edges sorted by (window, tgt) -> slot arrays
        e_win = win_of[e_src]
        e_r = r_of[e_src]
        eorder = np.argsort(e_win, kind="stable")
        e_win_s = e_win[eorder]
        e_r_s = e_r[eorder]
        e_tgt_s = e_tgt[eorder]

        srel_arr = np.full(cfg.ntiles * P, 255.0, np.float32)
        gidx_arr = np.zeros(cfg.ntiles * P, np.int64)
        wstart = np.searchsorted(e_win_s, np.arange(nwin + 1))
        for w in range(nwin):
            a, b = wstart[w], wstart[w + 1]
            n = b - a
            assert n <= cap_e
            s0 = w * cap_e
            srel_arr[s0 : s0 + n] = e_r_s[a:b]
            gidx_arr[s0 : s0 + n] = e_tgt_s[a:b] - IDX_OFF

        # per gather chunk: last slot must have idx >= 0 (trailing negative
        # indices are trimmed by the ucode). Swap inside the last tile.
        cs = cfg.tpc * P  # slots per chunk
        for ch in range(cfg.nchunks):
            last = (ch + 1) * cs - 1
            if gidx_arr[last] < 0:
                tile0 = last - P + 1
                cand = np.nonzero(gidx_arr[tile0 : last + 1] >= 0)[0]
                assert len(cand) > 0, "all-negative last tile"
                j = tile0 + cand[0]
                gidx_arr[last], gidx_arr[j] = gidx_arr[j], gidx_arr[last]
                srel_arr[last], srel_arr[j] = srel_arr[j], srel_arr[last]

        plan.srel.append(
            np.ascontiguousarray(srel_arr.reshape(cfg.ntiles, P).T.astype(BF16))
        )
        g16 = gidx_arr.astype(np.int16).reshape(cfg.ntiles * 8, 16).T  # [16, nt*8]
        plan.gidx.append(np.ascontiguousarray(np.tile(g16, (8, 1)).astype(np.int16)))

        # xs padded: [P, nwin*F], row r of window w = input[src with (w,r)]
        xs = np.zeros((nwin * P, F), np.float32)
        rows = win_of * P + r_of  # padded row per source
        xs[rows] = inp[c * npc : (c + 1) * npc]
        plan.xs.append(
            np.ascontiguousarray(
                xs.reshape(nwin, P, F).transpose(1, 0, 2).reshape(P, nwin * F)
            )
        )
        plan.inv_rows.append(rows.copy())
    return plan


def _host_arrays(cfg: Cfg, inp: np.ndarray, W: np.ndarray):
    n = cfg.n_nodes
    xT = np.zeros((P, cfg.na * P), np.float32)
    xT[:, :n] = inp.T
    wT = np.ascontiguousarray(W.T)  # [F, 2F]
    iota = np.tile(np.arange(P, dtype=np.float32), (P, 1))
    return (
        np.ascontiguousarray(xT.astype(BF16)),
        np.ascontiguousarray(wT.astype(BF16)),
        np.ascontiguousarray(iota.astype(BF16)),
    )


def _build(cfg: Cfg, enable_asserts: bool = False):
    import concourse.bacc as bacc
    import concourse.tile as tile
    from concourse import mybir
    from concourse.tile import add_dep_helper

    nc = bacc.Bacc(
        "TRN2",
        target_bir_lowering=False,
        debug=False,
        enable_asserts=enable_asserts,
        num_devices=cfg.n_cores,
        num_swdge_queues=4,
    )
    dt = mybir.dt

    na, nwin, tpw, tpc = cfg.na, cfg.nwin, cfg.tpw, cfg.tpc
    ntiles, nchunks, ca = cfg.ntiles, cfg.nchunks, cfg.ca

    xT_d = nc.dram_tensor("xT", [P, na * P], dt.bfloat16, kind="ExternalInput")
    wT_d = nc.dram_tensor("wT", [P, TF], dt.bfloat16, kind="ExternalInput")
    iota_d = nc.dram_tensor("iota", [P, P], dt.bfloat16, kind="ExternalInput")
    xs_d = nc.dram_tensor("xs", [P, nwin * F], dt.float32, kind="ExternalInput")
    srel_d = nc.dram_tensor("srel", [P, ntiles], dt.bfloat16, kind="ExternalInput")
    gidx_d = nc.dram_tensor("gidx", [P, 8 * ntiles], dt.int16, kind="ExternalInput")
    y_d = nc.dram_tensor("y", [nwin * P, F], dt.float32, kind="ExternalOutput")
    # M table at 512B row stride (payload in first 256B of each row)
    mtab_d = nc.dram_tensor("mtab", [na * P, 2 * F], dt.bfloat16, kind="Internal")

    n_achunks = math.ceil(na / ca)

    with tile.TileContext(nc) as tc:
        import contextlib

        with contextlib.ExitStack() as ctx:
            consts = ctx.enter_context(tc.tile_pool(name="consts", bufs=1))
            a_in = ctx.enter_context(tc.tile_pool(name="a_in", bufs=3))
            a_ps = ctx.enter_context(tc.tile_pool(name="a_ps", bufs=2, space="PSUM"))
            a_sg = ctx.enter_context(tc.tile_pool(name="a_sg", bufs=3))
            a_m = ctx.enter_context(tc.tile_pool(name="a_m", bufs=3))
            b_g = ctx.enter_context(tc.tile_pool(name="b_g", bufs=4))
            b_oh = ctx.enter_context(tc.tile_pool(name="b_oh", bufs=2))
            b_ps = ctx.enter_context(tc.tile_pool(name="b_ps", bufs=4, space="PSUM"))
            b_out = ctx.enter_context(tc.tile_pool(name="b_out", bufs=3))

            # ---- constants to SBUF ----
            wT_sb = consts.tile([P, TF], dt.bfloat16, tag="wT")
            nc.sync.dma_start(wT_sb[:], wT_d[:, :])
            iota_sb = consts.tile([P, P], dt.bfloat16, tag="iota")
            nc.sync.dma_start(iota_sb[:], iota_d[:, :])
            xs_sb = consts.tile([P, nwin * F], dt.float32, tag="xs")
            nc.sync.dma_start(xs_sb[:], xs_d[:, :])
            srel_sb = consts.tile([P, ntiles], dt.bfloat16, tag="srel")
            nc.sync.dma_start(srel_sb[:], srel_d[:, :])
            gidx_sb = consts.tile([P, 8 * ntiles], dt.int16, tag="gidx")
            nc.sync.dma_start(gidx_sb[:], gidx_d[:, :])

            # ---- phase A: M table ----
            mdmas = []
            for ci in range(n_achunks):
                c0 = ci * ca
                cn = min(ca, na - c0)
                xt = a_in.tile([P, ca * P], dt.bfloat16, tag="xt")
                nc.sync.dma_start(xt[:, : cn * P], xT_d[:, c0 * P : (c0 + cn) * P])
                ps = a_ps.tile([P, ca * TF], dt.float32, tag="psA")
                for k in range(cn):
                    nc.tensor.matmul(
                        ps[:, k * TF : (k + 1) * TF],
                        lhsT=xt[:, k * P : (k + 1) * P],
                        rhs=wT_sb[:],
                        start=True,
                        stop=True,
                    )
                # sigmoid over the g-halves of all cn tiles in one op
                sg = a_sg.tile([P, ca * F], dt.float32, tag="sg")
                nc.scalar.activation(
                    sg[:, : cn * F].rearrange("p (k f) -> p k f", f=F),
                    ps[:, : cn * TF]
                    .rearrange("p (k f) -> p k f", f=TF)[:, :, 0:F],
                    mybir.ActivationFunctionType.Sigmoid,
                )
                mtile = a_m.tile([P, ca * F], dt.bfloat16, tag="mtile")
                nc.vector.tensor_mul(
                    mtile[:, : cn * F].rearrange("p (k f) -> p k f", f=F),
                    ps[:, : cn * TF]
                    .rearrange("p (k f) -> p k f", f=TF)[:, :, F:TF],
                    sg[:, : cn * F].rearrange("p (k f) -> p k f", f=F),
                )
                out_ap = (
                    mtab_d[c0 * P : (c0 + cn) * P, 0:F]
                    .rearrange("(k p) f -> p k f", p=P)
                )
                mdmas.append(
                    nc.sync.dma_start(
                        out_ap, mtile[:, : cn * F].rearrange("p (k f) -> p k f", f=F)
                    )
                )

            # ---- phase B: gather chunks (4 SWDGE queues) + one-hot scatter ----
            gbufs = []
            for ch in range(nchunks):
                t0 = ch * tpc
                gb = b_g.tile([P, tpc * F], dt.bfloat16, tag="gb")
                g = nc.gpsimd.dma_gather(
                    out_ap=gb[:].rearrange("p (t e) -> p t e", e=F),
                    in_ap=mtab_d[IDX_OFF : na * P, 0:F],
                    idxs_ap=gidx_sb[:, 8 * t0 : 8 * (t0 + tpc)],
                    num_idxs=tpc * P,
                    num_idxs_reg=tpc * P,
                    elem_size=F,
                    elem_step=2 * F,
                    single_packet=False,
                    queue_num=ch % 4,
                )
                for m in mdmas:
                    add_dep_helper(g.ins, m.ins, reason="mtab RAW")
                oh = b_oh.tile([P, tpc * P], dt.bfloat16, tag="oh")
                nc.vector.tensor_tensor(
                    out=oh[:].rearrange("p (t e) -> p t e", e=P),
                    in0=srel_sb[:, t0 : t0 + tpc]
                    .unsqueeze(2)
                    .to_broadcast([P, tpc, P]),
                    in1=iota_sb[:].unsqueeze(1).to_broadcast([P, tpc, P]),
                    op=mybir.AluOpType.is_equal,
                )
                gbufs.append((gb, oh))

            for w in range(nwin):
                ps = b_ps.tile([P, F], dt.float32, tag="psB")
                for i in range(tpw):
                    t = w * tpw + i
                    ch, pos = divmod(t, tpc)
                    gb, oh = gbufs[ch]
                    nc.tensor.matmul(
                        ps[:],
                        lhsT=oh[:, pos * P : (pos + 1) * P],
                        rhs=gb[:, pos * F : (pos + 1) * F],
                        start=(i == 0),
                        stop=(i == tpw - 1),
                    )
                ot = b_out.tile([P, F], dt.float32, tag="ot")
                nc.vector.tensor_add(ot[:], ps[:], xs_sb[:, w * F : (w + 1) * F])
                nc.sync.dma_start(y_d[w * P : (w + 1) * P, :], ot[:])

    nc.compile()
    return nc


def _in_maps(cfg: Cfg, plan: Plan, inp: np.ndarray, W: np.ndarray):
    xT, wT, iota = _host_arrays(cfg, inp, W)
    maps = []
    for c in range(cfg.n_cores):
        maps.append(
            {
                "xT": xT,
                "wT": wT,
                "iota": iota,
                "xs": plan.xs[c],
                "srel": plan.srel[c],
                "gidx": plan.gidx[c],
            }
        )
    return maps


def _install_ntff_hook():
    """Provide the antenv.axon_hooks shim trn_boot expects, so trace=True
    can capture NTFF profiles. Silently degrades if anything is missing."""
    try:
        import antenv.axon_hooks  # noqa: F401

        return
    except ImportError:
        pass
    try:
        import types

        import antenv

        mod = types.ModuleType("antenv.axon_hooks")
        _hook = [None]
        mod.set_axon_ntff_profile_hook = lambda h: _hook.__setitem__(0, h)
        mod.get_axon_ntff_profile_hook = lambda: _hook[0]
        sys.modules["antenv.axon_hooks"] = mod
        antenv.axon_hooks = mod
        from trn_agent_boot import trn_boot

        mod.set_axon_ntff_profile_hook(
            trn_boot._ntff_profile_via_ctypes("/opt/axon/libaxon_pjrt.so")
        )
    except Exception:
        pass


def kernel(**inputs) -> np.ndarray:
    inp = np.asarray(inputs["input"], np.float32)
    W = np.asarray(inputs["W"], np.float32)
    es = np.asarray(inputs["edge_sources"]).astype(np.int64)
    et = np.asarray(inputs["edge_targets"]).astype(np.int64)

    cfg = Cfg(n_nodes=inp.shape[0])
    plan = _plan(cfg, es, et, inp)
    nc = _build(cfg)

    from concourse.bass_utils import run_bass_kernel_spmd

    if bool(int(os.environ.get("GGC_TRACE", "0"))):
        _install_ntff_hook()
    res = run_bass_kernel_spmd(
        nc,
        _in_maps(cfg, plan, inp, W),
        core_ids=list(range(cfg.n_cores)),
        trace=bool(int(os.environ.get("GGC_TRACE", "0"))),
    )
    out = np.empty((cfg.n_nodes, F), np.float32)
    for c in range(cfg.n_cores):
        y = np.asarray(res.results[c]["y"])  # [nwin*P, F] padded
        out[c * cfg.npc : (c + 1) * cfg.npc] = y[plan.inv_rows[c]]
    if bool(int(os.environ.get("GGC_TRACE", "0"))):
        kernel.last_results = res  # stash for test harness
    return out


# revision 6
# speedup vs baseline: 1.6412x; 1.0233x over previous
"""GatedGraphConvolution Trainium2 kernel (v2).

out = input + segment_sum(sigmoid(g) * e, edge_sources)
  where [g|e] = input[edge_targets] @ W.T

Per-edge messages depend only on the target node:
  msg_e = M[target_e],  M[n] = sigmoid(x_n @ Wg.T) * (x_n @ We.T)
Phase A computes the dense M table once per core (replicated) into DRAM at a
512-byte row stride; phase B gathers M rows per edge with gpsimd dma_gather
and scatter-adds by source via one-hot matmuls accumulated in PSUM.

v2 speedups over v1:
 - dma_gather calls rotate over 4 SWDGE queues (descriptor rings on all four
   Q7 core pairs) -> ~3x gather throughput vs one queue.
 - single index space: mtab rows at 512B stride, signed int16 idx = tgt-25024
   (covers all 50048 rows; negative idx reach below the AP base stays inside
   the tensor). The v1 low/high-half split and its padding are gone.
 - per-core sources are bin-packed into 51 balanced windows of <=128 sources
   and exactly 16 edge-tiles (2048 edge slots); the host permutation is
   undone after the run. Uniform structure = shared SPMD program, ~10% fewer
   gathered rows than v1.
 - phase A sigmoid/mul run as chunk-wide strided ops (fewer, larger DVE/ACT
   instructions).
"""

import math
import os
import sys
from dataclasses import dataclass, field
from heapq import heapify, heappop, heappush

import numpy as np

if "/opt/trn_rl_repo" not in sys.path:
    sys.path.insert(0, "/opt/trn_rl_repo")

import ml_dtypes

P = 128  # partitions / tile edge
F = 128  # feature dim (OUT_F == IN_F == 128)
TF = 2 * F

BF16 = ml_dtypes.bfloat16

IDX_OFF = 25024  # gather idx = tgt - IDX_OFF, fits int16 for tgt in [0, 50048)


@dataclass
class Cfg:
    n_nodes: int = 50000
    n_cores: int = 8
    nwin: int = 51  # windows per core (each <=128 sources, 16 tiles)
    tpw: int = 16  # tiles per window
    tpc: int = 51  # tiles per gather chunk
    ca: int = 4  # node-tiles per phase-A chunk

    @property
    def na(self) -> int:  # node tiles in M table
        return math.ceil(self.n_nodes / P)

    @property
    def npc(self) -> int:  # nodes per core
        assert self.n_nodes % self.n_cores == 0
        return self.n_nodes // self.n_cores

    @property
    def ntiles(self) -> int:
        return self.nwin * self.tpw

    @property
    def nchunks(self) -> int:
        assert self.ntiles % self.tpc == 0
        return self.ntiles // self.tpc


@dataclass
class Plan:
    srel: list = field(default_factory=list)  # per-core [P, ntiles] bf16
    gidx: list = field(default_factory=list)  # per-core [P, 8*ntiles] int16
    xs: list = field(default_factory=list)  # per-core [P, nwin*F] fp32
    inv_rows: list = field(default_factory=list)  # per-core [npc] row in padded y


def _plan(cfg: Cfg, edge_sources: np.ndarray, edge_targets: np.ndarray, inp: np.ndarray) -> Plan:
    src = edge_sources.astype(np.int64)
    tgt = edge_targets.astype(np.int64)
    npc, nwin, tpw = cfg.npc, cfg.nwin, cfg.tpw
    cap_e = tpw * P  # 2048 edge slots per window
    plan = Plan()

    core = src // npc
    for c in range(cfg.n_cores):
        m = core == c
        e_src = (src[m] - c * npc).astype(np.int64)
        e_tgt = tgt[m]
        deg = np.bincount(e_src, minlength=npc)

        # LPT bin-packing: sources (heaviest first) onto windows, respecting
        # <=128 sources and <=2048 edges per window.
        order = np.argsort(-deg, kind="stable")
        heap = [(0, 0, w) for w in range(nwin)]  # (edges, nsrc, w)
        heapify(heap)
        win_of = np.empty(npc, np.int32)
        r_of = np.empty(npc, np.int32)
        overflow = []
        for s in order:
            d = int(deg[s])
            spill = []
            while True:
                e_w, n_w, w = heappop(heap)
                if n_w < P and e_w + d <= cap_e:
                    win_of[s] = w
                    r_of[s] = n_w
                    heappush(heap, (e_w + d, n_w + 1, w))
                    break
                spill.append((e_w, n_w, w))
                if not heap:
                    raise RuntimeError("binpack failed")
            for it in spill:
                heappush(heap, it)

        # edges sorted by (window, tgt) -> slot arrays
        e_win = win_of[e_src]
        e_r = r_of[e_src]
        eorder = np.argsort(e_win, kind="stable")
        e_win_s = e_win[eorder]
        e_r_s = e_r[eorder]
        e_tgt_s = e_tgt[eorder]

        srel_arr = np.full(cfg.ntiles * P, 255.0, np.float32)
        gidx_arr = np.zeros(cfg.ntiles * P, np.int64)
        wstart = np.searchsorted(e_win_s, np.arange(nwin + 1))
        for w in range(nwin):
            a, b = wstart[w], wstart[w + 1]
            n = b - a
            assert n <= cap_e
            s0 = w * cap_e
            srel_arr[s0 : s0 + n] = e_r_s[a:b]
            gidx_arr[s0 : s0 + n] = e_tgt_s[a:b] - IDX_OFF

        # per gather chunk: last slot must have idx >= 0 (trailing negative
        # indices are trimmed by the ucode). Swap inside the last tile.
        cs = cfg.tpc * P  # slots per chunk
        for ch in range(cfg.nchunks):
            last = (ch + 1) * cs - 1
            if gidx_arr[last] < 0:
                tile0 = last - P + 1
                cand = np.nonzero(gidx_arr[tile0 : last + 1] >= 0)[0]
                assert len(cand) > 0, "all-negative last tile"
                j = tile0 + cand[0]
                gidx_arr[last], gidx_arr[j] = gidx_arr[j], gidx_arr[last]
                srel_arr[last], srel_arr[j] = srel_arr[j], srel_arr[last]

        plan.srel.append(
            np.ascontiguousarray(srel_arr.reshape(cfg.ntiles, P).T.astype(BF16))
        )
        g16 = gidx_arr.astype(np.int16).reshape(cfg.ntiles * 8, 16).T  # [16, nt*8]
        plan.gidx.append(np.ascontiguousarray(np.tile(g16, (8, 1)).astype(np.int16)))

        # xs padded: [P, nwin*F], row r of window w = input[src with (w,r)]
        xs = np.zeros((nwin * P, F), np.float32)
        rows = win_of * P + r_of  # padded row per source
        xs[rows] = inp[c * npc : (c + 1) * npc]
        plan.xs.append(
            np.ascontiguousarray(
                xs.reshape(nwin, P, F).transpose(1, 0, 2).reshape(P, nwin * F)
            )
        )
        plan.inv_rows.append(rows.copy())
    return plan


def _host_arrays(cfg: Cfg, inp: np.ndarray, W: np.ndarray):
    n = cfg.n_nodes
    xT = np.zeros((P, cfg.na * P), np.float32)
    xT[:, :n] = inp.T
    wT = np.ascontiguousarray(W.T)  # [F, 2F]
    iota = np.tile(np.arange(P, dtype=np.float32), (P, 1))
    return (
        np.ascontiguousarray(xT.astype(BF16)),
        np.ascontiguousarray(wT.astype(BF16)),
        np.ascontiguousarray(iota.astype(BF16)),
    )


def _build(cfg: Cfg, enable_asserts: bool = False):
    import concourse.bacc as bacc
    import concourse.tile as tile
    from concourse import mybir
    from concourse.tile import add_dep_helper

    nc = bacc.Bacc(
        "TRN2",
        target_bir_lowering=False,
        debug=False,
        enable_asserts=enable_asserts,
        num_devices=cfg.n_cores,
        num_swdge_queues=4,
    )
    dt = mybir.dt

    na, nwin, tpw, tpc = cfg.na, cfg.nwin, cfg.tpw, cfg.tpc
    ntiles, nchunks, ca = cfg.ntiles, cfg.nchunks, cfg.ca

    xT_d = nc.dram_tensor("xT", [P, na * P], dt.bfloat16, kind="ExternalInput")
    wT_d = nc.dram_tensor("wT", [P, TF], dt.bfloat16, kind="ExternalInput")
    iota_d = nc.dram_tensor("iota", [P, P], dt.bfloat16, kind="ExternalInput")
    xs_d = nc.dram_tensor("xs", [P, nwin * F], dt.float32, kind="ExternalInput")
    srel_d = nc.dram_tensor("srel", [P, ntiles], dt.bfloat16, kind="ExternalInput")
    gidx_d = nc.dram_tensor("gidx", [P, 8 * ntiles], dt.int16, kind="ExternalInput")
    y_d = nc.dram_tensor("y", [nwin * P, F], dt.float32, kind="ExternalOutput")
    # M table at 512B row stride (payload in first 256B of each row)
    mtab_d = nc.dram_tensor("mtab", [na * P, 2 * F], dt.bfloat16, kind="Internal")

    n_achunks = math.ceil(na / ca)

    with tile.TileContext(nc) as tc:
        import contextlib

        with contextlib.ExitStack() as ctx:
            consts = ctx.enter_context(tc.tile_pool(name="consts", bufs=1))
            a_in = ctx.enter_context(tc.tile_pool(name="a_in", bufs=3))
            a_ps = ctx.enter_context(tc.tile_pool(name="a_ps", bufs=2, space="PSUM"))
            a_sg = ctx.enter_context(tc.tile_pool(name="a_sg", bufs=3))
            a_m = ctx.enter_context(tc.tile_pool(name="a_m", bufs=3))
            b_g = ctx.enter_context(tc.tile_pool(name="b_g", bufs=6))
            b_oh = ctx.enter_context(tc.tile_pool(name="b_oh", bufs=3))
            b_ps = ctx.enter_context(tc.tile_pool(name="b_ps", bufs=4, space="PSUM"))
            b_out = ctx.enter_context(tc.tile_pool(name="b_out", bufs=3))

            # ---- constants to SBUF ----
            wT_sb = consts.tile([P, TF], dt.bfloat16, tag="wT")
            nc.sync.dma_start(wT_sb[:], wT_d[:, :])
            iota_sb = consts.tile([P, P], dt.bfloat16, tag="iota")
            nc.sync.dma_start(iota_sb[:], iota_d[:, :])
            xs_sb = consts.tile([P, nwin * F], dt.float32, tag="xs")
            nc.sync.dma_start(xs_sb[:], xs_d[:, :])
            srel_sb = consts.tile([P, ntiles], dt.bfloat16, tag="srel")
            nc.sync.dma_start(srel_sb[:], srel_d[:, :])
            gidx_sb = consts.tile([P, 8 * ntiles], dt.int16, tag="gidx")
            nc.sync.dma_start(gidx_sb[:], gidx_d[:, :])

            # ---- phase A: M table ----
            mdmas = []
            for ci in range(n_achunks):
                c0 = ci * ca
                cn = min(ca, na - c0)
                xt = a_in.tile([P, ca * P], dt.bfloat16, tag="xt")
                nc.sync.dma_start(xt[:, : cn * P], xT_d[:, c0 * P : (c0 + cn) * P])
                ps = a_ps.tile([P, ca * TF], dt.float32, tag="psA")
                for k in range(cn):
                    nc.tensor.matmul(
                        ps[:, k * TF : (k + 1) * TF],
                        lhsT=xt[:, k * P : (k + 1) * P],
                        rhs=wT_sb[:],
                        start=True,
                        stop=True,
                    )
                # sigmoid over the g-halves of all cn tiles in one op
                sg = a_sg.tile([P, ca * F], dt.float32, tag="sg")
                nc.scalar.activation(
                    sg[:, : cn * F].rearrange("p (k f) -> p k f", f=F),
                    ps[:, : cn * TF]
                    .rearrange("p (k f) -> p k f", f=TF)[:, :, 0:F],
                    mybir.ActivationFunctionType.Sigmoid,
                )
                mtile = a_m.tile([P, ca * F], dt.bfloat16, tag="mtile")
                nc.vector.tensor_mul(
                    mtile[:, : cn * F].rearrange("p (k f) -> p k f", f=F),
                    ps[:, : cn * TF]
                    .rearrange("p (k f) -> p k f", f=TF)[:, :, F:TF],
                    sg[:, : cn * F].rearrange("p (k f) -> p k f", f=F),
                )
                out_ap = (
                    mtab_d[c0 * P : (c0 + cn) * P, 0:F]
                    .rearrange("(k p) f -> p k f", p=P)
                )
                mdmas.append(
                    nc.sync.dma_start(
                        out_ap, mtile[:, : cn * F].rearrange("p (k f) -> p k f", f=F)
                    )
                )

            # ---- phase B: gather chunks (4 SWDGE queues) + one-hot scatter ----
            nidx_reg = nc.gpsimd.to_reg(tpc * P)  # hoisted: avoid per-gather MOVE
            gbufs = []
            for ch in range(nchunks):
                t0 = ch * tpc
                gb = b_g.tile([P, tpc * F], dt.bfloat16, tag="gb")
                g = nc.gpsimd.dma_gather(
                    out_ap=gb[:].rearrange("p (t e) -> p t e", e=F),
                    in_ap=mtab_d[IDX_OFF : na * P, 0:F],
                    idxs_ap=gidx_sb[:, 8 * t0 : 8 * (t0 + tpc)],
                    num_idxs=tpc * P,
                    num_idxs_reg=nidx_reg,
                    elem_size=F,
                    elem_step=2 * F,
                    single_packet=False,
                    queue_num=ch % 4,
                )
                for m in mdmas:
                    add_dep_helper(g.ins, m.ins, reason="mtab RAW")
                oh = b_oh.tile([P, tpc * P], dt.bfloat16, tag="oh")
                nc.vector.tensor_tensor(
                    out=oh[:].rearrange("p (t e) -> p t e", e=P),
                    in0=srel_sb[:, t0 : t0 + tpc]
                    .unsqueeze(2)
                    .to_broadcast([P, tpc, P]),
                    in1=iota_sb[:].unsqueeze(1).to_broadcast([P, tpc, P]),
                    op=mybir.AluOpType.is_equal,
                )
                gbufs.append((gb, oh))

            for w in range(nwin):
                ps = b_ps.tile([P, F], dt.float32, tag="psB")
                for i in range(tpw):
                    t = w * tpw + i
                    ch, pos = divmod(t, tpc)
                    gb, oh = gbufs[ch]
                    nc.tensor.matmul(
                        ps[:],
                        lhsT=oh[:, pos * P : (pos + 1) * P],
                        rhs=gb[:, pos * F : (pos + 1) * F],
                        start=(i == 0),
                        stop=(i == tpw - 1),
                    )
                ot = b_out.tile([P, F], dt.float32, tag="ot")
                nc.vector.tensor_add(ot[:], ps[:], xs_sb[:, w * F : (w + 1) * F])
                nc.sync.dma_start(y_d[w * P : (w + 1) * P, :], ot[:])

    nc.compile()
    return nc


def _in_maps(cfg: Cfg, plan: Plan, inp: np.ndarray, W: np.ndarray):
    xT, wT, iota = _host_arrays(cfg, inp, W)
    maps = []
    for c in range(cfg.n_cores):
        maps.append(
            {
                "xT": xT,
                "wT": wT,
                "iota": iota,
                "xs": plan.xs[c],
                "srel": plan.srel[c],
                "gidx": plan.gidx[c],
            }
        )
    return maps


def _install_ntff_hook():
    """Provide the antenv.axon_hooks shim trn_boot expects, so trace=True
    can capture NTFF profiles. Silently degrades if anything is missing."""
    try:
        import antenv.axon_hooks  # noqa: F401

        return
    except ImportError:
        pass
    try:
        import types

        import antenv

        mod = types.ModuleType("antenv.axon_hooks")
        _hook = [None]
        mod.set_axon_ntff_profile_hook = lambda h: _hook.__setitem__(0, h)
        mod.get_axon_ntff_profile_hook = lambda: _hook[0]
        sys.modules["antenv.axon_hooks"] = mod
        antenv.axon_hooks = mod
        from trn_agent_boot import trn_boot

        mod.set_axon_ntff_profile_hook(
            trn_boot._ntff_profile_via_ctypes("/opt/axon/libaxon_pjrt.so")
        )
    except Exception:
        pass


def kernel(**inputs) -> np.ndarray:
    inp = np.asarray(inputs["input"], np.float32)
    W = np.asarray(inputs["W"], np.float32)
    es = np.asarray(inputs["edge_sources"]).astype(np.int64)
    et = np.asarray(inputs["edge_targets"]).astype(np.int64)

    cfg = Cfg(n_nodes=inp.shape[0])
    plan = _plan(cfg, es, et, inp)
    nc = _build(cfg)

    from concourse.bass_utils import run_bass_kernel_spmd

    if bool(int(os.environ.get("GGC_TRACE", "0"))):
        _install_ntff_hook()
    res = run_bass_kernel_spmd(
        nc,
        _in_maps(cfg, plan, inp, W),
        core_ids=list(range(cfg.n_cores)),
        trace=bool(int(os.environ.get("GGC_TRACE", "0"))),
    )
    out = np.empty((cfg.n_nodes, F), np.float32)
    for c in range(cfg.n_cores):
        y = np.asarray(res.results[c]["y"])  # [nwin*P, F] padded
        out[c * cfg.npc : (c + 1) * cfg.npc] = y[plan.inv_rows[c]]
    if bool(int(os.environ.get("GGC_TRACE", "0"))):
        kernel.last_results = res  # stash for test harness
    return out


# revision 16
# speedup vs baseline: 1.8066x; 1.1008x over previous
"""GatedGraphConvolution Trainium2 kernel (v2).

out = input + segment_sum(sigmoid(g) * e, edge_sources)
  where [g|e] = input[edge_targets] @ W.T

Per-edge messages depend only on the target node:
  msg_e = M[target_e],  M[n] = sigmoid(x_n @ Wg.T) * (x_n @ We.T)
Phase A computes the dense M table once per core (replicated) into DRAM at a
512-byte row stride; phase B gathers M rows per edge with gpsimd dma_gather
and scatter-adds by source via one-hot matmuls accumulated in PSUM.

v2 speedups over v1:
 - dma_gather calls rotate over 4 SWDGE queues (descriptor rings on all four
   Q7 core pairs) -> ~3x gather throughput vs one queue.
 - single index space: mtab rows at 512B stride, signed int16 idx = tgt-25024
   (covers all 50048 rows; negative idx reach below the AP base stays inside
   the tensor). The v1 low/high-half split and its padding are gone.
 - per-core sources are bin-packed into 51 balanced windows of <=128 sources
   and exactly 16 edge-tiles (2048 edge slots); the host permutation is
   undone after the run. Uniform structure = shared SPMD program, ~10% fewer
   gathered rows than v1.
 - phase A sigmoid/mul run as chunk-wide strided ops (fewer, larger DVE/ACT
   instructions).
"""

import math
import os
import sys
from dataclasses import dataclass, field
from heapq import heapify, heappop, heappush

import numpy as np

if "/opt/trn_rl_repo" not in sys.path:
    sys.path.insert(0, "/opt/trn_rl_repo")

import ml_dtypes

P = 128  # partitions / tile edge
F = 128  # feature dim (OUT_F == IN_F == 128)
TF = 2 * F

BF16 = ml_dtypes.bfloat16

IDX_OFF = 25024  # gather idx = tgt - IDX_OFF, fits int16 for tgt in [0, 50048)


@dataclass
class Cfg:
    n_nodes: int = 50000
    n_cores: int = 8
    nwin: int = 51  # windows per core (each <=128 sources, 16 tiles)
    tpw: int = 16  # tiles per window
    tpc: int = 51  # tiles per gather chunk
    ca: int = 4  # node-tiles per phase-A chunk

    @property
    def na(self) -> int:  # node tiles in M table
        return math.ceil(self.n_nodes / P)

    @property
    def npc(self) -> int:  # nodes per core
        assert self.n_nodes % self.n_cores == 0
        return self.n_nodes // self.n_cores

    @property
    def ntiles(self) -> int:
        return self.nwin * self.tpw

    @property
    def nchunks(self) -> int:
        assert self.ntiles % self.tpc == 0
        return self.ntiles // self.tpc


@dataclass
class Plan:
    srel: list = field(default_factory=list)  # per-core [P, ntiles] bf16
    gidx: list = field(default_factory=list)  # per-core [P, 8*ntiles] int16
    xs: list = field(default_factory=list)  # per-core [P, nwin*F] fp32
    inv_rows: list = field(default_factory=list)  # per-core [npc] row in padded y
    xTc: list = field(default_factory=list)  # per-core compacted [P, na_c*P] bf16
    na_c: int = 0  # node tiles in the compacted M table (max over cores)


def _plan(cfg: Cfg, edge_sources: np.ndarray, edge_targets: np.ndarray, inp: np.ndarray) -> Plan:
    src = edge_sources.astype(np.int64)
    tgt = edge_targets.astype(np.int64)
    npc, nwin, tpw = cfg.npc, cfg.nwin, cfg.tpw
    cap_e = tpw * P  # 2048 edge slots per window
    plan = Plan()

    core = src // npc
    # compact the M table per core to the targets it actually gathers
    uniqs, invs = [], []
    for c in range(cfg.n_cores):
        u, iv = np.unique(tgt[core == c], return_inverse=True)
        uniqs.append(u)
        invs.append(iv)
    plan.na_c = math.ceil(max(len(u) for u in uniqs) / P)
    center = (plan.na_c * P) // 2
    assert plan.na_c * P - center <= 32767 and center <= 32768

    for c in range(cfg.n_cores):
        m = core == c
        e_src = (src[m] - c * npc).astype(np.int64)
        e_tgt = invs[c]  # compacted target ids
        deg = np.bincount(e_src, minlength=npc)
        xTc = np.zeros((P, plan.na_c * P), np.float32)
        xTc[:, : len(uniqs[c])] = inp[uniqs[c]].T
        plan.xTc.append(np.ascontiguousarray(xTc.astype(BF16)))

        # LPT bin-packing: sources (heaviest first) onto windows, respecting
        # <=128 sources and <=2048 edges per window.
        order = np.argsort(-deg, kind="stable")
        heap = [(0, 0, w) for w in range(nwin)]  # (edges, nsrc, w)
        heapify(heap)
        win_of = np.empty(npc, np.int32)
        r_of = np.empty(npc, np.int32)
        overflow = []
        for s in order:
            d = int(deg[s])
            spill = []
            while True:
                e_w, n_w, w = heappop(heap)
                if n_w < P and e_w + d <= cap_e:
                    win_of[s] = w
                    r_of[s] = n_w
                    heappush(heap, (e_w + d, n_w + 1, w))
                    break
                spill.append((e_w, n_w, w))
                if not heap:
                    raise RuntimeError("binpack failed")
            for it in spill:
                heappush(heap, it)

        # edges sorted by (window, tgt) -> slot arrays
        e_win = win_of[e_src]
        e_r = r_of[e_src]
        eorder = np.argsort(e_win, kind="stable")
        e_win_s = e_win[eorder]
        e_r_s = e_r[eorder]
        e_tgt_s = e_tgt[eorder]

        srel_arr = np.full(cfg.ntiles * P, 255.0, np.float32)
        gidx_arr = np.zeros(cfg.ntiles * P, np.int64)
        wstart = np.searchsorted(e_win_s, np.arange(nwin + 1))
        for w in range(nwin):
            a, b = wstart[w], wstart[w + 1]
            n = b - a
            assert n <= cap_e
            s0 = w * cap_e
            srel_arr[s0 : s0 + n] = e_r_s[a:b]
            gidx_arr[s0 : s0 + n] = e_tgt_s[a:b] - center

        # per gather chunk: last slot must have idx >= 0 (trailing negative
        # indices are trimmed by the ucode). Swap inside the last tile.
        cs = cfg.tpc * P  # slots per chunk
        for ch in range(cfg.nchunks):
            last = (ch + 1) * cs - 1
            if gidx_arr[last] < 0:
                tile0 = last - P + 1
                cand = np.nonzero(gidx_arr[tile0 : last + 1] >= 0)[0]
                assert len(cand) > 0, "all-negative last tile"
                j = tile0 + cand[0]
                gidx_arr[last], gidx_arr[j] = gidx_arr[j], gidx_arr[last]
                srel_arr[last], srel_arr[j] = srel_arr[j], srel_arr[last]

        plan.srel.append(
            np.ascontiguousarray(srel_arr.reshape(cfg.ntiles, P).T.astype(BF16))
        )
        g16 = gidx_arr.astype(np.int16).reshape(cfg.ntiles * 8, 16).T  # [16, nt*8]
        plan.gidx.append(np.ascontiguousarray(np.tile(g16, (8, 1)).astype(np.int16)))

        # xs padded: [P, nwin*F], row r of window w = input[src with (w,r)]
        xs = np.zeros((nwin * P, F), np.float32)
        rows = win_of * P + r_of  # padded row per source
        xs[rows] = inp[c * npc : (c + 1) * npc]
        plan.xs.append(
            np.ascontiguousarray(
                xs.reshape(nwin, P, F).transpose(1, 0, 2).reshape(P, nwin * F)
            )
        )
        plan.inv_rows.append(rows.copy())
    return plan


def _host_arrays(cfg: Cfg, W: np.ndarray):
    wT = np.ascontiguousarray(W.T)  # [F, 2F]
    iota = np.tile(np.arange(P, dtype=np.float32), (P, 1))
    return (
        np.ascontiguousarray(wT.astype(BF16)),
        np.ascontiguousarray(iota.astype(BF16)),
    )


def _build(cfg: Cfg, na_c: int, enable_asserts: bool = False):
    import concourse.bacc as bacc
    import concourse.tile as tile
    from concourse import mybir
    from concourse.tile import add_dep_helper

    nc = bacc.Bacc(
        "TRN2",
        target_bir_lowering=False,
        debug=False,
        enable_asserts=enable_asserts,
        num_devices=cfg.n_cores,
        num_swdge_queues=4,
    )
    dt = mybir.dt

    na = na_c  # compacted table size
    center = (na * P) // 2
    nwin, tpw, tpc = cfg.nwin, cfg.tpw, cfg.tpc
    ntiles, nchunks, ca = cfg.ntiles, cfg.nchunks, cfg.ca

    xT_d = nc.dram_tensor("xT", [P, na * P], dt.bfloat16, kind="ExternalInput")
    wT_d = nc.dram_tensor("wT", [P, TF], dt.bfloat16, kind="ExternalInput")
    iota_d = nc.dram_tensor("iota", [P, P], dt.bfloat16, kind="ExternalInput")
    xs_d = nc.dram_tensor("xs", [P, nwin * F], dt.float32, kind="ExternalInput")
    srel_d = nc.dram_tensor("srel", [P, ntiles], dt.bfloat16, kind="ExternalInput")
    gidx_d = nc.dram_tensor("gidx", [P, 8 * ntiles], dt.int16, kind="ExternalInput")
    y_d = nc.dram_tensor("y", [nwin * P, F], dt.float32, kind="ExternalOutput")
    # M table at 512B row stride (payload in first 256B of each row)
    mtab_d = nc.dram_tensor("mtab", [na * P, 2 * F], dt.bfloat16, kind="Internal")

    n_achunks = math.ceil(na / ca)

    with tile.TileContext(nc) as tc:
        import contextlib

        with contextlib.ExitStack() as ctx:
            consts = ctx.enter_context(tc.tile_pool(name="consts", bufs=1))
            a_in = ctx.enter_context(tc.tile_pool(name="a_in", bufs=3))
            a_ps = ctx.enter_context(tc.tile_pool(name="a_ps", bufs=2, space="PSUM"))
            a_sg = ctx.enter_context(tc.tile_pool(name="a_sg", bufs=3))
            a_m = ctx.enter_context(tc.tile_pool(name="a_m", bufs=3))
            b_g = ctx.enter_context(tc.tile_pool(name="b_g", bufs=6))
            b_oh = ctx.enter_context(tc.tile_pool(name="b_oh", bufs=3))
            b_ps = ctx.enter_context(tc.tile_pool(name="b_ps", bufs=4, space="PSUM"))
            b_out = ctx.enter_context(tc.tile_pool(name="b_out", bufs=3))

            # ---- constants to SBUF ----
            wT_sb = consts.tile([P, TF], dt.bfloat16, tag="wT")
            nc.sync.dma_start(wT_sb[:], wT_d[:, :])
            iota_sb = consts.tile([P, P], dt.bfloat16, tag="iota")
            nc.sync.dma_start(iota_sb[:], iota_d[:, :])
            xs_sb = consts.tile([P, nwin * F], dt.float32, tag="xs")
            nc.sync.dma_start(xs_sb[:], xs_d[:, :])
            srel_sb = consts.tile([P, ntiles], dt.bfloat16, tag="srel")
            nc.sync.dma_start(srel_sb[:], srel_d[:, :])
            gidx_sb = consts.tile([P, 8 * ntiles], dt.int16, tag="gidx")
            nc.sync.dma_start(gidx_sb[:], gidx_d[:, :])

            # dummy gather at t=0: pulls the gpsimd library IRAM load +
            # MODIFY_POOL_CONFIG off the critical path (reads uninitialized
            # mtab, result discarded; no deps so it schedules immediately)
            pre = consts.tile([P, F], dt.bfloat16, tag="pre")
            nc.gpsimd.dma_gather(
                out_ap=pre[:].rearrange("p (t e) -> p t e", e=F),
                in_ap=mtab_d[center : na * P, 0:F],
                idxs_ap=gidx_sb[:, 0:8],
                num_idxs=P,
                num_idxs_reg=P,
                elem_size=F,
                elem_step=2 * F,
                single_packet=False,
                queue_num=0,
            )

            # ---- phase A: M table ----
            mdmas = []
            for ci in range(n_achunks):
                c0 = ci * ca
                cn = min(ca, na - c0)
                xt = a_in.tile([P, ca * P], dt.bfloat16, tag="xt")
                nc.sync.dma_start(xt[:, : cn * P], xT_d[:, c0 * P : (c0 + cn) * P])
                ps = a_ps.tile([P, ca * TF], dt.float32, tag="psA")
                for k in range(cn):
                    nc.tensor.matmul(
                        ps[:, k * TF : (k + 1) * TF],
                        lhsT=xt[:, k * P : (k + 1) * P],
                        rhs=wT_sb[:],
                        start=True,
                        stop=True,
                    )
                # sigmoid over the g-halves of all cn tiles in one op
                sg = a_sg.tile([P, ca * F], dt.float32, tag="sg")
                nc.scalar.activation(
                    sg[:, : cn * F].rearrange("p (k f) -> p k f", f=F),
                    ps[:, : cn * TF]
                    .rearrange("p (k f) -> p k f", f=TF)[:, :, 0:F],
                    mybir.ActivationFunctionType.Sigmoid,
                )
                mtile = a_m.tile([P, ca * F], dt.bfloat16, tag="mtile")
                nc.vector.tensor_mul(
                    mtile[:, : cn * F].rearrange("p (k f) -> p k f", f=F),
                    ps[:, : cn * TF]
                    .rearrange("p (k f) -> p k f", f=TF)[:, :, F:TF],
                    sg[:, : cn * F].rearrange("p (k f) -> p k f", f=F),
                )
                out_ap = (
                    mtab_d[c0 * P : (c0 + cn) * P, 0:F]
                    .rearrange("(k p) f -> p k f", p=P)
                )
                mdmas.append(
                    nc.sync.dma_start(
                        out_ap, mtile[:, : cn * F].rearrange("p (k f) -> p k f", f=F)
                    )
                )

            # ---- phase B: gather chunks (4 SWDGE queues) + one-hot scatter ----
            nidx_reg = nc.gpsimd.to_reg(tpc * P)  # hoisted: avoid per-gather MOVE
            gbufs = []
            for ch in range(nchunks):
                t0 = ch * tpc
                gb = b_g.tile([P, tpc * F], dt.bfloat16, tag="gb")
                g = nc.gpsimd.dma_gather(
                    out_ap=gb[:].rearrange("p (t e) -> p t e", e=F),
                    in_ap=mtab_d[center : na * P, 0:F],
                    idxs_ap=gidx_sb[:, 8 * t0 : 8 * (t0 + tpc)],
                    num_idxs=tpc * P,
                    num_idxs_reg=nidx_reg,
                    elem_size=F,
                    elem_step=2 * F,
                    single_packet=False,
                    queue_num=ch % 4,
                )
                for m in mdmas:
                    add_dep_helper(g.ins, m.ins, reason="mtab RAW")
                oh = b_oh.tile([P, tpc * P], dt.bfloat16, tag="oh")
                nc.vector.tensor_tensor(
                    out=oh[:].rearrange("p (t e) -> p t e", e=P),
                    in0=srel_sb[:, t0 : t0 + tpc]
                    .unsqueeze(2)
                    .to_broadcast([P, tpc, P]),
                    in1=iota_sb[:].unsqueeze(1).to_broadcast([P, tpc, P]),
                    op=mybir.AluOpType.is_equal,
                )
                gbufs.append((gb, oh))

            for w in range(nwin):
                ps = b_ps.tile([P, F], dt.float32, tag="psB")
                for i in range(tpw):
                    t = w * tpw + i
                    ch, pos = divmod(t, tpc)
                    gb, oh = gbufs[ch]
                    nc.tensor.matmul(
                        ps[:],
                        lhsT=oh[:, pos * P : (pos + 1) * P],
                        rhs=gb[:, pos * F : (pos + 1) * F],
                        start=(i == 0),
                        stop=(i == tpw - 1),
                    )
                ot = b_out.tile([P, F], dt.float32, tag="ot")
                nc.vector.tensor_add(ot[:], ps[:], xs_sb[:, w * F : (w + 1) * F])
                nc.sync.dma_start(y_d[w * P : (w + 1) * P, :], ot[:])

    nc.compile()
    return nc


def _in_maps(cfg: Cfg, plan: Plan, inp: np.ndarray, W: np.ndarray):
    wT, iota = _host_arrays(cfg, W)
    maps = []
    for c in range(cfg.n_cores):
        maps.append(
            {
                "xT": plan.xTc[c],
                "wT": wT,
                "iota": iota,
                "xs": plan.xs[c],
                "srel": plan.srel[c],
                "gidx": plan.gidx[c],
            }
        )
    return maps


def _install_ntff_hook():
    """Provide the antenv.axon_hooks shim trn_boot expects, so trace=True
    can capture NTFF profiles. Silently degrades if anything is missing."""
    try:
        import antenv.axon_hooks  # noqa: F401

        return
    except ImportError:
        pass
    try:
        import types

        import antenv

        mod = types.ModuleType("antenv.axon_hooks")
        _hook = [None]
        mod.set_axon_ntff_profile_hook = lambda h: _hook.__setitem__(0, h)
        mod.get_axon_ntff_profile_hook = lambda: _hook[0]
        sys.modules["antenv.axon_hooks"] = mod
        antenv.axon_hooks = mod
        from trn_agent_boot import trn_boot

        mod.set_axon_ntff_profile_hook(
            trn_boot._ntff_profile_via_ctypes("/opt/axon/libaxon_pjrt.so")
        )
    except Exception:
        pass


def kernel(**inputs) -> np.ndarray:
    inp = np.asarray(inputs["input"], np.float32)
    W = np.asarray(inputs["W"], np.float32)
    es = np.asarray(inputs["edge_sources"]).astype(np.int64)
    et = np.asarray(inputs["edge_targets"]).astype(np.int64)

    cfg = Cfg(n_nodes=inp.shape[0])
    plan = _plan(cfg, es, et, inp)
    nc = _build(cfg, plan.na_c)

    from concourse.bass_utils import run_bass_kernel_spmd

    if bool(int(os.environ.get("GGC_TRACE", "0"))):
        _install_ntff_hook()
    res = run_bass_kernel_spmd(
        nc,
        _in_maps(cfg, plan, inp, W),
        core_ids=list(range(cfg.n_cores)),
        trace=bool(int(os.environ.get("GGC_TRACE", "0"))),
    )
    out = np.empty((cfg.n_nodes, F), np.float32)
    for c in range(cfg.n_cores):
        y = np.asarray(res.results[c]["y"])  # [nwin*P, F] padded
        out[c * cfg.npc : (c + 1) * cfg.npc] = y[plan.inv_rows[c]]
    if bool(int(os.environ.get("GGC_TRACE", "0"))):
        kernel.last_results = res  # stash for test harness
    return out
